# revision 2
# baseline (speedup 1.0000x reference)
"""CTRNN cell (6 Euler unfolds) on 8 Trainium2 NeuronCores.

Math (per unfold, 6x):
    f     = tanh([x, s] @ W + b)
    s_new = s + 0.1 * (-s + f)  = 0.9*s + 0.1*f

Strategy (v2 — bf16 rewrite of the f32r baseline):
  - Data-parallel over batch: B=8192 -> 1024 rows/core, no cross-core
    communication. Host does the cheap numpy transposes/packing/casts.
  - All tensors kept TRANSPOSED on-chip (feature dim on SBUF partitions,
    batch on the free dim): state/x feed the tensor engine as the moving
    operand and W k-tile slices are directly the stationary lhsT.
  - Everything bf16 except the running state v (f32, so 5 accumulating
    updates never round at bf16 ULP) and the PSUM pre-activations.
    Measured end-to-end rel err ~2e-3 (gate is 2e-2).
  - Delta-form matmuls: one persistent PSUM accumulator per (m-tile,
    chunk) holds z = x@Wt + s_k@Wb across all unfolds, updated with
    psum += (f_k - s_k) @ (0.1*Wb). 7-logical-matmul FLOP floor; PSUM
    never restarts. 0.1*Wb is host-prefolded; the state is kept scaled
    (v = 10*s) so the init matmul v0 @ (0.1*Wb) == s0@Wb and the state
    update is the plain add v += tmp.
  - Batch is split into 2 chunks of 512 that alternate on the PE: while
    chunk A runs its 16 delta matmuls, chunk B does tanh (ACT) + the
    tmp = f - 0.1*v fused op (DVE) for the same unfold, and vice versa.
    The PE never waits out an ACT->DVE dependency chain, so it stays
    busy (and the HAM clock gate stays at 2.4 GHz) for the whole
    steady-state region.
  - State adds v += tmp run 1x on DVE and 3x on GpSimd per round, off
    the critical path; tanh ACT folds the bias via the per-partition
    bias operand; bf16 tanh output feeds the matmuls directly (no cast).
  - Inputs are host-packed (128, k*4KB) chunk-major so every DMA has
    >=4KB-contiguous per-partition runs, spread over the sync/scalar
    HWDGE rings and the gpsimd SWDGE path.
  - A junk-matmul warm-up keeps the PE activity monitor from throttling
    the clock during the input-load phase.
"""

import numpy as np
import ml_dtypes

UNFOLDS = 6
DT = 0.1
B, D, N = 8192, 512, 512
NCORES = 8
BC = B // NCORES          # batch rows per core (1024)
CH = 512                  # chunk free-dim (one PSUM bank of f32)
NCH = BC // CH            # 2 chunks
P = 128
KT = D // P               # 4 k-tiles per operand half
MT = N // P               # 4 m-tiles of the output dim

_compiled_nc = None


def _build_nc():
    import concourse.bass as bass  # noqa: F401
    import concourse.bacc as bacc
    import concourse.tile as tile
    from concourse import mybir

    f32 = mybir.dt.float32
    bf16 = mybir.dt.bfloat16
    MULT = mybir.AluOpType.mult
    ADD = mybir.AluOpType.add
    TANH = mybir.ActivationFunctionType.Tanh

    nc = bacc.Bacc("TRN2", target_bir_lowering=False, debug=False)

    xP = nc.dram_tensor("xP", [P, NCH * KT * CH], bf16, kind="ExternalInput").ap()
    vP = nc.dram_tensor("vP", [P, NCH * KT * CH], bf16, kind="ExternalInput").ap()
    WP = nc.dram_tensor("WP", [P, 2 * KT * N], bf16, kind="ExternalInput").ap()
    bias = nc.dram_tensor("bias", [N], f32, kind="ExternalInput").ap()
    outT = nc.dram_tensor("outT", [N, BC], f32, kind="ExternalOutput").ap()

    with tile.TileContext(nc) as tc:
        with (
            tc.tile_pool(name="wpool", bufs=1) as wpool,
            tc.tile_pool(name="data", bufs=1) as data,
            tc.tile_pool(name="fpool", bufs=2) as fpool,
            tc.tile_pool(name="tmpp", bufs=2) as tmpp,
            tc.tile_pool(name="psum", bufs=1, space="PSUM") as psump,
        ):
            # warm-up junk tile first thing on the gpsimd queue so the
            # warm-up matmuls can start before any data lands
            junk = wpool.tile([P, N], bf16, tag="junk", name="junk")
            nc.gpsimd.memset(junk[:], 0)

            # ---- input DMAs --------------------------------------------
            # first-needed first: Wt + x_A feed the first init matmuls.
            w_sb = wpool.tile([P, 2 * KT * N], bf16, tag="w", name="w_sb")
            nc.scalar.dma_start(w_sb[:, 0:KT * N], WP[:, 0:KT * N])
            x_sb = data.tile([P, NCH * KT * CH], bf16, tag="x", name="x_sb")
            nc.sync.dma_start(x_sb[:, 0:KT * CH], xP[:, 0:KT * CH])
            v0b = data.tile([P, NCH * KT * CH], bf16, tag="v0", name="v0b")
            nc.gpsimd.dma_start(v0b[:, 0:KT * CH], vP[:, 0:KT * CH])
            nc.sync.dma_start(x_sb[:, KT * CH:], xP[:, KT * CH:])
            nc.scalar.dma_start(w_sb[:, KT * N:], WP[:, KT * N:])
            nc.gpsimd.dma_start(v0b[:, KT * CH:], vP[:, KT * CH:])
            bias_sb = wpool.tile([P, MT], f32, tag="bias", name="bias_sb")
            nc.sync.dma_start(bias_sb[:], bias.rearrange("(m p) -> p m", p=P))

            wt = [w_sb[:, j * N:(j + 1) * N] for j in range(KT)]
            wb01 = [w_sb[:, (KT + j) * N:(KT + j + 1) * N] for j in range(KT)]

            def xs(c, j):
                return x_sb[:, (c * KT + j) * CH:(c * KT + j + 1) * CH]

            def v0s(c, j):
                return v0b[:, (c * KT + j) * CH:(c * KT + j + 1) * CH]

            # f32 running state (scaled: v = 10*s)
            vf = data.tile([P, NCH * KT * CH], f32, tag="vf", name="vf")

            def vfs(c, m):
                return vf[:, (c * KT + m) * CH:(c * KT + m + 1) * CH]

            for c in range(NCH):
                for m in range(MT):
                    if m % 2 == 0:
                        nc.vector.tensor_copy(vfs(c, m), v0s(c, m))
                    else:
                        nc.scalar.copy(vfs(c, m), v0s(c, m))

            # persistent PSUM accumulators: ps[m][:, c*CH:...] is one bank
            ps = [psump.tile([P, BC], f32, tag=f"ps{m}", name=f"ps{m}")
                  for m in range(MT)]

            # HAM warm-up: keep the PE busy while inputs stream in so real
            # matmuls run at 2.4 GHz from the start. Overwritten by the
            # first start=True matmul per bank.
            for r in range(20):
                nc.tensor.matmul(
                    ps[r % MT][:, 0:CH],
                    lhsT=junk[:, 0:P], rhs=junk[:, 0:CH],
                    start=True, stop=True, skip_group_check=True,
                )

            def mm_round(weights, rhs_of_j, c, start, stop):
                for j in range(KT):
                    for m in range(MT):
                        nc.tensor.matmul(
                            ps[m][:, c * CH:(c + 1) * CH],
                            lhsT=weights[j][:, m * P:(m + 1) * P],
                            rhs=rhs_of_j(j),
                            start=(start and j == 0),
                            stop=(stop and j == KT - 1),
                            skip_group_check=True,
                        )

            # init: psum = x @ Wt + v0 @ (0.1*Wb)   (== x@Wt + s0@Wb)
            for c in range(NCH):
                mm_round(wt, lambda j: xs(c, j), c, start=True, stop=False)
                mm_round(wb01, lambda j: v0s(c, j), c, start=False, stop=False)

            # ---- unfolds: chunks alternate on the PE -------------------
            for k in range(UNFOLDS):
                last = k == UNFOLDS - 1
                for c in range(NCH):
                    f_t = [fpool.tile([P, CH], bf16, tag=f"f{c}_{m}",
                                      name=f"f{k}_{c}_{m}")
                           for m in range(MT)]
                    if not last:
                        tmp_t = [tmpp.tile([P, CH], bf16, tag=f"t{c}_{m}",
                                           name=f"t{k}_{c}_{m}")
                                 for m in range(MT)]
                        for m in range(MT):
                            # f = tanh(psum + bias), bf16 out feeds the PE
                            nc.scalar.activation(
                                f_t[m][:], ps[m][:, c * CH:(c + 1) * CH],
                                TANH, bias=bias_sb[:, m:m + 1], scale=1.0,
                            )
                            # tmp = f - 0.1*v  (== f - s)
                            nc.vector.scalar_tensor_tensor(
                                tmp_t[m][:], vfs(c, m), -DT, f_t[m][:],
                                op0=MULT, op1=ADD,
                            )
                        # psum += tmp @ (0.1*Wb)
                        mm_round(wb01, lambda j: tmp_t[j][:], c,
                                 start=False, stop=(k == UNFOLDS - 2))
                        # v += tmp (off critical path; mostly on GpSimd)
                        for m in range(MT):
                            eng = nc.vector if m == 0 else nc.gpsimd
                            eng.tensor_tensor(vfs(c, m), vfs(c, m),
                                              tmp_t[m][:], ADD)
                    else:
                        # final unfold: s_out = 0.1*(0.9*v + f)
                        for m in range(MT):
                            nc.scalar.activation(
                                f_t[m][:], ps[m][:, c * CH:(c + 1) * CH],
                                TANH, bias=bias_sb[:, m:m + 1], scale=1.0,
                            )
                            w10 = tmpp.tile([P, CH], f32, tag=f"o{c}_{m}",
                                            name=f"o{c}_{m}")
                            nc.vector.scalar_tensor_tensor(
                                w10[:], vfs(c, m), 0.9, f_t[m][:],
                                op0=MULT, op1=ADD,
                            )
                            nc.vector.tensor_scalar_mul(vfs(c, m), w10[:], DT)
                            out_eng = (nc.sync, nc.scalar, nc.gpsimd,
                                       nc.sync)[m]
                            out_eng.dma_start(
                                outT[m * P:(m + 1) * P, c * CH:(c + 1) * CH],
                                vfs(c, m))

    nc.compile()
    return nc


def _get_nc():
    global _compiled_nc
    if _compiled_nc is None:
        _compiled_nc = _build_nc()
    return _compiled_nc


def _pack_cm(a):
    """(512, 1024) f32 -> (128, NCH*KT*CH) bf16, chunk-major (c, j)."""
    t = a.reshape(KT, P, NCH, CH).transpose(1, 2, 0, 3).reshape(P, -1)
    return np.ascontiguousarray(t).astype(ml_dtypes.bfloat16)


def make_in_maps(x, s, W, b):
    xT = np.ascontiguousarray(x.T)           # (D, B)
    sT = np.ascontiguousarray(s.T)           # (N, B)
    Wt = W[:D].reshape(KT, P, N).transpose(1, 0, 2).reshape(P, -1)
    Wb01 = (DT * W[D:]).reshape(KT, P, N).transpose(1, 0, 2).reshape(P, -1)
    WPh = np.ascontiguousarray(
        np.concatenate([Wt, Wb01], axis=1)).astype(ml_dtypes.bfloat16)
    in_maps = []
    for c in range(NCORES):
        sl = slice(c * BC, (c + 1) * BC)
        in_maps.append({
            "xP": _pack_cm(xT[:, sl]),
            "vP": _pack_cm(10.0 * sT[:, sl]),
            "WP": WPh,
            "bias": b,
        })
    return in_maps


def kernel(**inputs):
    from concourse.bass_utils import run_bass_kernel_spmd

    x = np.asarray(inputs["inputs"], dtype=np.float32)
    s = np.asarray(inputs["state"], dtype=np.float32)
    W = np.ascontiguousarray(np.asarray(inputs["W"], dtype=np.float32))
    b = np.ascontiguousarray(np.asarray(inputs["bias"], dtype=np.float32))

    in_maps = make_in_maps(x, s, W, b)
    nc = _get_nc()
    res = run_bass_kernel_spmd(nc, in_maps, list(range(NCORES))).results
    outT = np.concatenate([res[c]["outT"] for c in range(NCORES)], axis=1)
    out = np.ascontiguousarray(outT.T).astype(np.float32)
    return (out, out)


# revision 3
# speedup vs baseline: 1.3136x; 1.3136x over previous
"""CTRNN cell (6 Euler unfolds) on 8 Trainium2 NeuronCores.

Math (per unfold, 6x):
    f     = tanh([x, s] @ W + b)
    s_new = s + 0.1 * (-s + f)  = 0.9*s + 0.1*f

Strategy (v3 — fp16 rewrite):
  - Data-parallel over batch: B=8192 -> 1024 rows/core, no cross-core
    communication. Host does the cheap numpy transposes/packing/casts.
  - All tensors kept TRANSPOSED on-chip (feature dim on SBUF partitions,
    batch on the free dim): state/x feed the tensor engine as the moving
    operand and W k-tile slices are directly the stationary lhsT.
  - Everything fp16 (except PSUM + final output f32). fp16 runs the PE
    at bf16 rate, gives every DVE op the 2x packed perf mode, and its
    10-bit mantissa keeps the 5 accumulating state updates accurate:
    simulated end-to-end rel err 1.2e-3 (gate is 2e-2).
  - Delta-form matmuls: one persistent PSUM bank per (m-tile, chunk)
    holds z = x@Wt + s_k@Wb across all unfolds, updated with
    psum += (f_k - s_k) @ (0.1*Wb). 7-logical-matmul FLOP floor; PSUM
    never restarts. 0.1*Wb is host-prefolded; the state is kept scaled
    (v = 10*s) so the init matmul v0 @ (0.1*Wb) == s0@Wb and the state
    update is the plain add v += tmp.
  - Batch is split into 2 chunks of 512 that alternate on the PE: while
    chunk A runs its 16 delta matmuls, chunk B does tanh (ACT) + the
    tmp = f - 0.1*v fused op (DVE) for the same unfold, and vice versa.
    The PE never waits out the ACT->DVE chain, so it stays busy (and the
    HAM clock gate stays at 2.4 GHz) through the steady state. All
    PSUM/state tiles are split per (m-tile, chunk) so Tile's
    tile-granular hazard tracking cannot serialize the two chunks.
  - Per round DVE does 4 stt (tmp) + 4 adds (state), all 16-bit 2x ops
    (~3.3us per 2-chunk round) — under the PE's 7us. GpSimd only runs
    SWDGE input DMAs. Tanh ACT folds the bias via the per-partition
    bias operand; fp16 tanh output feeds the matmuls directly.
  - Inputs are host-packed (128, k*4KB) chunk-major so every DMA has
    >=4KB-contiguous per-partition runs, spread over the sync/scalar
    HWDGE rings and the gpsimd SWDGE path.
  - A short junk-matmul warm-up keeps the PE activity monitor from
    throttling the clock during the input-load phase without delaying
    the first real matmuls.
"""

import numpy as np

UNFOLDS = 6
DT = 0.1
B, D, N = 8192, 512, 512
NCORES = 8
BC = B // NCORES          # batch rows per core (1024)
CH = 512                  # chunk free-dim (one PSUM bank of f32)
NCH = BC // CH            # 2 chunks
P = 128
KT = D // P               # 4 k-tiles per operand half
MT = N // P               # 4 m-tiles of the output dim

_compiled_nc = None


def _build_nc():
    import concourse.bass as bass  # noqa: F401
    import concourse.bacc as bacc
    import concourse.tile as tile
    from concourse import mybir

    f32 = mybir.dt.float32
    f16 = mybir.dt.float16
    MULT = mybir.AluOpType.mult
    ADD = mybir.AluOpType.add
    TANH = mybir.ActivationFunctionType.Tanh

    nc = bacc.Bacc("TRN2", target_bir_lowering=False, debug=False)

    xP = nc.dram_tensor("xP", [P, NCH * KT * CH], f16, kind="ExternalInput").ap()
    vP = nc.dram_tensor("vP", [P, NCH * KT * CH], f16, kind="ExternalInput").ap()
    WP = nc.dram_tensor("WP", [P, 2 * KT * N], f16, kind="ExternalInput").ap()
    bias = nc.dram_tensor("bias", [N], f32, kind="ExternalInput").ap()
    outT = nc.dram_tensor("outT", [N, BC], f32, kind="ExternalOutput").ap()

    with tile.TileContext(nc) as tc:
        with (
            tc.tile_pool(name="wpool", bufs=1) as wpool,
            tc.tile_pool(name="data", bufs=1) as data,
            tc.tile_pool(name="fpool", bufs=2) as fpool,
            tc.tile_pool(name="tmpp", bufs=2) as tmpp,
            tc.tile_pool(name="outp", bufs=1) as outp,
            tc.tile_pool(name="psum", bufs=1, space="PSUM") as psump,
        ):
            # warm-up junk tile first thing on the gpsimd queue so the
            # warm-up matmuls can start before any data lands
            junk = wpool.tile([P, N], f16, tag="junk", name="junk")
            nc.gpsimd.memset(junk[:], 0)

            # ---- input DMAs --------------------------------------------
            # first-needed first: Wt + x_A feed the first init matmuls.
            w_sb = wpool.tile([P, 2 * KT * N], f16, tag="w", name="w_sb")
            nc.scalar.dma_start(w_sb[:, 0:KT * N], WP[:, 0:KT * N])
            x_sb = data.tile([P, NCH * KT * CH], f16, tag="x", name="x_sb")
            nc.sync.dma_start(x_sb[:, 0:KT * CH], xP[:, 0:KT * CH])
            # state v = 10*s, one tile per chunk (updated in place)
            v_t = [data.tile([P, KT * CH], f16, tag=f"v{c}", name=f"v{c}")
                   for c in range(NCH)]
            nc.gpsimd.dma_start(v_t[0][:], vP[:, 0:KT * CH])
            nc.sync.dma_start(x_sb[:, KT * CH:], xP[:, KT * CH:])
            nc.scalar.dma_start(w_sb[:, KT * N:], WP[:, KT * N:])
            nc.gpsimd.dma_start(v_t[1][:], vP[:, KT * CH:])
            bias_sb = wpool.tile([P, MT], f32, tag="bias", name="bias_sb")
            nc.sync.dma_start(bias_sb[:], bias.rearrange("(m p) -> p m", p=P))

            wt = [w_sb[:, j * N:(j + 1) * N] for j in range(KT)]
            wb01 = [w_sb[:, (KT + j) * N:(KT + j + 1) * N] for j in range(KT)]

            def xs(c, j):
                return x_sb[:, (c * KT + j) * CH:(c * KT + j + 1) * CH]

            def vs(c, m):
                return v_t[c][:, m * CH:(m + 1) * CH]

            # one PSUM bank per (m-tile, chunk) so chunk hazards stay
            # independent under tile-granular tracking
            ps = [[psump.tile([P, CH], f32, tag=f"ps{m}_{c}",
                              name=f"ps{m}_{c}")
                   for c in range(NCH)] for m in range(MT)]

            # HAM warm-up (short: must not delay the first real matmul)
            for r in range(8):
                nc.tensor.matmul(
                    ps[r % MT][0][:],
                    lhsT=junk[:, 0:P], rhs=junk[:, 0:CH],
                    start=True, stop=True, skip_group_check=True,
                )

            def mm_round(weights, rhs_of_j, c, start, stop):
                for j in range(KT):
                    for m in range(MT):
                        nc.tensor.matmul(
                            ps[m][c][:],
                            lhsT=weights[j][:, m * P:(m + 1) * P],
                            rhs=rhs_of_j(j),
                            start=(start and j == 0),
                            stop=(stop and j == KT - 1),
                            skip_group_check=True,
                        )

            # init: psum = x @ Wt + v0 @ (0.1*Wb)   (== x@Wt + s0@Wb)
            for c in range(NCH):
                mm_round(wt, lambda j: xs(c, j), c, start=True, stop=False)
                mm_round(wb01, lambda j: vs(c, j), c, start=False, stop=False)

            # ---- unfolds: chunks alternate on the PE -------------------
            for k in range(UNFOLDS):
                last = k == UNFOLDS - 1
                for c in range(NCH):
                    f_t = [fpool.tile([P, CH], f16, tag=f"f{c}_{m}",
                                      name=f"f{k}_{c}_{m}")
                           for m in range(MT)]
                    if not last:
                        tmp_t = [tmpp.tile([P, CH], f16, tag=f"t{c}_{m}",
                                           name=f"t{k}_{c}_{m}")
                                 for m in range(MT)]
                        for m in range(MT):
                            # f = tanh(psum + bias), fp16 out feeds the PE
                            nc.scalar.activation(
                                f_t[m][:], ps[m][c][:],
                                TANH, bias=bias_sb[:, m:m + 1], scale=1.0,
                            )
                            # tmp = f - 0.1*v  (== f - s)
                            nc.vector.scalar_tensor_tensor(
                                tmp_t[m][:], vs(c, m), -DT, f_t[m][:],
                                op0=MULT, op1=ADD,
                            )
                        # psum += tmp @ (0.1*Wb)
                        mm_round(wb01, lambda j: tmp_t[j][:], c,
                                 start=False, stop=(k == UNFOLDS - 2))
                        # v += tmp (2x fp16 adds, off the critical path)
                        for m in range(MT):
                            nc.vector.tensor_tensor(vs(c, m), vs(c, m),
                                                    tmp_t[m][:], ADD)
                    else:
                        # final unfold: s_out = 0.1*(0.9*v + f)
                        for m in range(MT):
                            nc.scalar.activation(
                                f_t[m][:], ps[m][c][:],
                                TANH, bias=bias_sb[:, m:m + 1], scale=1.0,
                            )
                            w10 = tmpp.tile([P, CH], f16, tag=f"t{c}_{m}",
                                            name=f"w10_{c}_{m}")
                            nc.vector.scalar_tensor_tensor(
                                w10[:], vs(c, m), 0.9, f_t[m][:],
                                op0=MULT, op1=ADD,
                            )
                            o_t = outp.tile([P, CH], f32, tag=f"o{c}_{m}",
                                            name=f"o{c}_{m}")
                            nc.vector.tensor_scalar_mul(o_t[:], w10[:], DT)
                            out_eng = (nc.sync, nc.scalar, nc.gpsimd,
                                       nc.sync)[m]
                            out_eng.dma_start(
                                outT[m * P:(m + 1) * P, c * CH:(c + 1) * CH],
                                o_t[:])

    nc.compile()
    return nc


def _get_nc():
    global _compiled_nc
    if _compiled_nc is None:
        _compiled_nc = _build_nc()
    return _compiled_nc


def _pack_cm(a):
    """(512, 1024) f32 -> (128, NCH*KT*CH) fp16, chunk-major (c, j)."""
    t = a.reshape(KT, P, NCH, CH).transpose(1, 2, 0, 3).reshape(P, -1)
    return np.ascontiguousarray(t).astype(np.float16)


def make_in_maps(x, s, W, b):
    xT = np.ascontiguousarray(x.T)           # (D, B)
    sT = np.ascontiguousarray(s.T)           # (N, B)
    Wt = W[:D].reshape(KT, P, N).transpose(1, 0, 2).reshape(P, -1)
    Wb01 = (DT * W[D:]).reshape(KT, P, N).transpose(1, 0, 2).reshape(P, -1)
    WPh = np.ascontiguousarray(
        np.concatenate([Wt, Wb01], axis=1)).astype(np.float16)
    in_maps = []
    for c in range(NCORES):
        sl = slice(c * BC, (c + 1) * BC)
        in_maps.append({
            "xP": _pack_cm(xT[:, sl]),
            "vP": _pack_cm(10.0 * sT[:, sl]),
            "WP": WPh,
            "bias": b,
        })
    return in_maps


def kernel(**inputs):
    from concourse.bass_utils import run_bass_kernel_spmd

    x = np.asarray(inputs["inputs"], dtype=np.float32)
    s = np.asarray(inputs["state"], dtype=np.float32)
    W = np.ascontiguousarray(np.asarray(inputs["W"], dtype=np.float32))
    b = np.ascontiguousarray(np.asarray(inputs["bias"], dtype=np.float32))

    in_maps = make_in_maps(x, s, W, b)
    nc = _get_nc()
    res = run_bass_kernel_spmd(nc, in_maps, list(range(NCORES))).results
    outT = np.concatenate([res[c]["outT"] for c in range(NCORES)], axis=1)
    out = np.ascontiguousarray(outT.T).astype(np.float32)
    return (out, out)
